# revision 1
# baseline (speedup 1.0000x reference)
"""GCN layer kernel for Trainium2 (8 NeuronCores).

out = relu(x @ U^T + segment_sum(x[src], dst) @ V^T)

Strategy: nodes are sharded row-wise across 8 cores; U, V replicated.
The edge aggregation (gather + segment-sum) is computed host-side as a
sparse CSR matmul over the adjacency matrix; each core then runs a Bass
kernel computing relu(U @ xT_c + V @ aggT_c) over its node shard in
512-column PSUM tiles with double-buffered DMA.
"""
import sys

sys.path.insert(0, "/opt/trn_rl_repo")

import numpy as np

from concourse import bacc, bass, mybir, tile
from concourse.bass_utils import run_bass_kernel_spmd

N_NODES = 50000
D = 64
N_CORES = 8
SHARD = N_NODES // N_CORES          # 6250 nodes per core
CHUNK = 512                         # PSUM bank free size in f32
NCHUNK = (SHARD + CHUNK - 1) // CHUNK   # 13
SHARD_PAD = NCHUNK * CHUNK          # 6656

_F32 = mybir.dt.float32


def _build_nc():
    nc = bacc.Bacc(None, target_bir_lowering=False)

    xT_d = nc.dram_tensor("xT", [D, SHARD_PAD], _F32, kind="ExternalInput")
    aggT_d = nc.dram_tensor("aggT", [D, SHARD_PAD], _F32, kind="ExternalInput")
    Ut_d = nc.dram_tensor("Ut", [D, D], _F32, kind="ExternalInput")
    Vt_d = nc.dram_tensor("Vt", [D, D], _F32, kind="ExternalInput")
    out_d = nc.dram_tensor("outT", [D, SHARD_PAD], _F32, kind="ExternalOutput")

    with tile.TileContext(nc) as tc:
        with (
            tc.tile_pool(name="w", bufs=1) as wpool,
            tc.tile_pool(name="io", bufs=4) as iopool,
            tc.tile_pool(name="ps", bufs=2, space=bass.MemorySpace.PSUM) as pspool,
        ):
            Ut_t = wpool.tile([D, D], _F32)
            nc.gpsimd.dma_start(Ut_t[:], Ut_d[:])
            Vt_t = wpool.tile([D, D], _F32)
            nc.gpsimd.dma_start(Vt_t[:], Vt_d[:])

            for i in range(NCHUNK):
                xt = iopool.tile([D, CHUNK], _F32)
                nc.gpsimd.dma_start(xt[:], xT_d[:, bass.ts(i, CHUNK)])
                at = iopool.tile([D, CHUNK], _F32)
                nc.gpsimd.dma_start(at[:], aggT_d[:, bass.ts(i, CHUNK)])

                ps = pspool.tile([D, CHUNK], _F32)
                # outT = Ut.T @ xT + Vt.T @ aggT = U @ xT + V @ aggT
                nc.tensor.matmul(ps[:], Ut_t[:], xt[:], start=True, stop=False)
                nc.tensor.matmul(ps[:], Vt_t[:], at[:], start=False, stop=True)

                ot = iopool.tile([D, CHUNK], _F32)
                nc.scalar.activation(ot[:], ps[:], mybir.ActivationFunctionType.Relu)
                nc.gpsimd.dma_start(out_d[:, bass.ts(i, CHUNK)], ot[:])

    nc.compile()
    return nc


_NC_CACHE = None


def _segment_sum(x: np.ndarray, src: np.ndarray, dst: np.ndarray) -> np.ndarray:
    src = np.asarray(src, dtype=np.int64)
    dst = np.asarray(dst, dtype=np.int64)
    try:
        from scipy.sparse import coo_matrix

        adj = coo_matrix(
            (np.ones(len(src), dtype=np.float32), (dst, src)),
            shape=(N_NODES, N_NODES),
        ).tocsr()
        return np.asarray(adj.dot(x), dtype=np.float32)
    except ImportError:
        order = np.argsort(dst, kind="stable")
        gathered = x[src[order]]
        dst_s = dst[order]
        starts = np.flatnonzero(np.r_[True, dst_s[1:] != dst_s[:-1]])
        sums = np.add.reduceat(gathered, starts, axis=0)
        agg = np.zeros((N_NODES, x.shape[1]), dtype=np.float32)
        agg[dst_s[starts]] = sums
        return agg


def kernel(x, src, dst, U, V):
    global _NC_CACHE
    x = np.ascontiguousarray(x, dtype=np.float32)
    U = np.ascontiguousarray(U, dtype=np.float32)
    V = np.ascontiguousarray(V, dtype=np.float32)

    agg = _segment_sum(x, src, dst)

    Ut = np.ascontiguousarray(U.T)
    Vt = np.ascontiguousarray(V.T)

    in_maps = []
    for c in range(N_CORES):
        lo, hi = c * SHARD, (c + 1) * SHARD
        xT = np.zeros((D, SHARD_PAD), dtype=np.float32)
        xT[:, :SHARD] = x[lo:hi].T
        aggT = np.zeros((D, SHARD_PAD), dtype=np.float32)
        aggT[:, :SHARD] = agg[lo:hi].T
        in_maps.append({"xT": xT, "aggT": aggT, "Ut": Ut, "Vt": Vt})

    if _NC_CACHE is None:
        _NC_CACHE = _build_nc()

    res = run_bass_kernel_spmd(_NC_CACHE, in_maps, core_ids=list(range(N_CORES)))

    out = np.empty((N_NODES, D), dtype=np.float32)
    for c in range(N_CORES):
        lo, hi = c * SHARD, (c + 1) * SHARD
        out[lo:hi] = res.results[c]["outT"][:, :SHARD].T
    return out



# revision 9
# speedup vs baseline: 2.6956x; 2.6956x over previous
"""GCN layer kernel for Trainium2 (8 NeuronCores).

out = relu(x @ U^T + segment_sum(x[src], dst) @ V^T)

Strategy: nodes (and output rows) are sharded across 8 cores; edges are
partitioned by destination core. Each core uploads only its x shard
(bf16); an on-device AllGather builds the full node-feature table. The
per-core edge list is gathered from that table with SWDGE dma_gather and
aggregated with dma_scatter_add into the core's agg shard; the dense
x@U^T + agg@V^T + ReLU runs on the tensor engine. The final output is
AllGathered on device so the host fetches a single core's shard.

Edge indices are int16 (SWDGE requirement), so gather sources are split
into lo (< 32768) and hi halves with an offset view of the table.
"""
import sys

sys.path.insert(0, "/opt/trn_rl_repo")

import numpy as np
import ml_dtypes

from concourse import bacc, bass, mybir, tile
from concourse.bass_utils import run_bass_kernel_spmd

N = 50000
D = 64
NCORES = 8
SHARD = N // NCORES          # 6250
SHARD_PAD = 6272             # 49 * 128
NFULL = NCORES * SHARD_PAD   # 50176
NT = SHARD_PAD // 128        # 49 row tiles per shard
SPLIT = 32768                # int16 gather range split
DUMMY = SHARD                # scatter target for padding edges (row 6250)

F32 = mybir.dt.float32
BF16 = mybir.dt.bfloat16
I16 = mybir.dt.int16
BF16_NP = ml_dtypes.bfloat16

# dma_scatter_add loses updates when one op carries duplicate indices, so
# edges are laid out in rounds: round k holds the k-th edge of every dst
# node, making indices within a scatter op unique (only the discarded
# DUMMY row sees duplicates). Static per-(round, src-half) capacities are
# sized from the Poisson(20) in-degree distribution with a fat margin.
P_LO = (5 * SHARD + SPLIT - 5 * SHARD_PAD) / N  # P(src_g < SPLIT) = 0.65316


def _poisson_sf(lam, kmax):
    """P(X > k) for k in 0..kmax-1."""
    import math
    pmf = [math.exp(-lam)]
    for i in range(1, kmax + 2):
        pmf.append(pmf[-1] * lam / i)
    sf, acc = [], 1.0
    for k in range(kmax):
        acc -= pmf[k]
        sf.append(max(acc, 0.0))
    return sf


def _make_caps():
    import math
    sf = _poisson_sf(1e6 / N, 96)
    caps = []  # per rank: (cap_lo, cap_hi)
    for k in range(96):
        mu = SHARD * sf[k]
        if mu < 1e-4 and k >= 40:
            break
        row = []
        for p in (P_LO, 1.0 - P_LO):
            m = mu * p
            cap = 128 * math.ceil((m + 6.0 * math.sqrt(max(m, 16.0)) + 16) / 128)
            row.append(cap)
        caps.append((row[0], row[1]))
    return caps


RANK_CAPS = _make_caps()
RMAX = len(RANK_CAPS)
# flat bucket layout per core: [(r0,lo) (r0,hi) (r1,lo) (r1,hi) ...]
BUCKET_OFF = []
_off = 0
for _lo, _hi in RANK_CAPS:
    BUCKET_OFF.append((_off, _off + _lo))
    _off += _lo + _hi
CAP = _off                   # padded edges per core


def _build_nc():
    nc = bacc.Bacc(None, target_bir_lowering=False, num_devices=NCORES)

    xb = nc.dram_tensor("xb", [SHARD_PAD, D], BF16, kind="ExternalInput")
    gidx = nc.dram_tensor("gidx", [16, CAP // 16], I16, kind="ExternalInput")
    sidx = nc.dram_tensor("sidx", [16, CAP // 16], I16, kind="ExternalInput")
    uv = nc.dram_tensor("uv", [2, D, D], BF16, kind="ExternalInput")
    outg = nc.dram_tensor("outg", [NFULL, D], BF16, kind="ExternalOutput")

    xs32 = nc.dram_tensor("xs32", [SHARD_PAD, D], F32)
    xf = nc.dram_tensor("xf", [NFULL, D], F32)
    agg = nc.dram_tensor("agg", [SHARD_PAD, D], F32)
    outl = nc.dram_tensor("outl", [SHARD_PAD, D], BF16)
    outgi = nc.dram_tensor("outgi", [NFULL, D], BF16)

    ident_d = nc.inline_tensor(np.eye(128, dtype=np.float32), name="ident")

    with tile.TileContext(nc) as tc:
        with (
            tc.tile_pool(name="w", bufs=1) as wpool,
            tc.tile_pool(name="cv", bufs=3) as cvpool,
            tc.tile_pool(name="g", bufs=2) as gpool,
            tc.tile_pool(name="d", bufs=3) as dpool,
            tc.tile_pool(name="psT", bufs=2, space=bass.MemorySpace.PSUM) as psT,
            tc.tile_pool(name="psO", bufs=2, space=bass.MemorySpace.PSUM) as psO,
        ):
            # idx tiles: [16, n] stripe replicated into all 8 gpsimd cores
            gidx_t = wpool.tile([128, CAP // 16], I16)
            sidx_t = wpool.tile([128, CAP // 16], I16)
            for r in range(8):
                nc.sync.dma_start(gidx_t[16 * r:16 * r + 16, :], gidx[:])
                nc.sync.dma_start(sidx_t[16 * r:16 * r + 16, :], sidx[:])
            ut_t = wpool.tile([D, D], BF16)
            nc.sync.dma_start(ut_t[:], uv[0])
            vt_t = wpool.tile([D, D], BF16)
            nc.sync.dma_start(vt_t[:], uv[1])
            id_t = wpool.tile([128, 128], F32)
            nc.sync.dma_start(id_t[:], ident_d[:])

            # own shard bf16 -> f32 (gather source must be 256B rows)
            for t in range(NT):
                bt = cvpool.tile([128, D], BF16)
                nc.sync.dma_start(bt[:], xb[t * 128:(t + 1) * 128, :])
                ft = cvpool.tile([128, D], F32)
                nc.scalar.copy(ft[:], bt[:])
                nc.sync.dma_start(xs32[t * 128:(t + 1) * 128, :], ft[:])

            # full f32 node table on every core
            nc.gpsimd.collective_compute(
                "AllGather", mybir.AluOpType.bypass,
                replica_groups=[list(range(NCORES))],
                ins=[xs32[:]], outs=[xf[:]])

            # zero the agg accumulator
            zt = wpool.tile([128, D], F32)
            nc.vector.memset(zt[:], 0.0)
            for t in range(NT):
                nc.sync.dma_start(agg[t * 128:(t + 1) * 128, :], zt[:])

            # edge aggregation in rounds: per round, gather lo+hi src halves
            # then scatter-add (dst unique within the round). Each SWDGE op
            # is capped at OPCAP indices: the descriptor ring holds
            # dynamic_dma_scratch_size/16 = 1024 descriptors and a single
            # op bigger than the ring deadlocks desc-gen.
            OPCAP = 1024
            max_span = max(lo + hi for lo, hi in RANK_CAPS)
            for r, (cap_lo, cap_hi) in enumerate(RANK_CAPS):
                off_lo, off_hi = BUCKET_OFF[r]
                span = cap_lo + cap_hi
                gt = gpool.tile([128, max_span // 128, D], F32)
                for base, cap, src_ap in (
                        (0, cap_lo, xf[0:SPLIT, :]),
                        (cap_lo, cap_hi, xf[SPLIT:NFULL, :])):
                    for o in range(0, cap, OPCAP):
                        sz = min(OPCAP, cap - o)
                        i0 = off_lo + base + o
                        nc.gpsimd.dma_gather(
                            gt[:, (base + o) // 128:(base + o + sz) // 128, :],
                            src_ap, gidx_t[:, i0 // 16:(i0 + sz) // 16],
                            sz, sz, D)
                for o in range(0, span, OPCAP):
                    sz = min(OPCAP, span - o)
                    i0 = off_lo + o
                    nc.gpsimd.dma_scatter_add(
                        agg[:], gt[:, o // 128:(o + sz) // 128, :],
                        sidx_t[:, i0 // 16:(i0 + sz) // 16], sz, sz, D)

            # dense phase: out = relu(x @ U^T + agg @ V^T), row-major tiles
            for t in range(NT):
                xt = dpool.tile([128, D], F32)
                nc.sync.dma_start(xt[:], xs32[t * 128:(t + 1) * 128, :])
                at = dpool.tile([128, D], F32)
                nc.sync.dma_start(at[:], agg[t * 128:(t + 1) * 128, :])
                pxT = psT.tile([D, 128], F32)
                nc.tensor.transpose(pxT[:], xt[:], id_t[:])
                xT = dpool.tile([D, 128], BF16)
                nc.scalar.copy(xT[:], pxT[:])
                paT = psT.tile([D, 128], F32)
                nc.tensor.transpose(paT[:], at[:], id_t[:])
                aT = dpool.tile([D, 128], BF16)
                nc.scalar.copy(aT[:], paT[:])
                po = psO.tile([128, D], F32)
                nc.tensor.matmul(po[:], xT[:], ut_t[:], start=True, stop=False)
                nc.tensor.matmul(po[:], aT[:], vt_t[:], start=False, stop=True)
                ot = dpool.tile([128, D], BF16)
                nc.scalar.activation(ot[:], po[:], mybir.ActivationFunctionType.Relu)
                nc.sync.dma_start(outl[t * 128:(t + 1) * 128, :], ot[:])

            # collect the full output on every core; host fetches one shard
            nc.gpsimd.collective_compute(
                "AllGather", mybir.AluOpType.bypass,
                replica_groups=[list(range(NCORES))],
                ins=[outl[:]], outs=[outgi[:]])
            nc.sync.dma_start(outg[:], outgi[:])

    nc.compile()
    return nc


def _counting_sort(key, nkeys):
    """Stable O(n) sort by small int key: returns (order, counts, starts)."""
    n = key.shape[0]
    try:
        from scipy.sparse import coo_matrix
        m = coo_matrix((np.ones(n, np.int8),
                        (key, np.arange(n, dtype=np.int32))),
                       shape=(nkeys, n)).tocsr()
        return m.indices, np.diff(m.indptr), m.indptr[:-1]
    except ImportError:
        order = np.argsort(key, kind="stable")
        counts = np.bincount(key, minlength=nkeys)
        starts = np.concatenate([[0], np.cumsum(counts)[:-1]])
        return order, counts, starts


_BUCKET_OFF_FLAT = None
_CAPS_FLAT = None


def _bucket_tables():
    global _BUCKET_OFF_FLAT, _CAPS_FLAT
    if _BUCKET_OFF_FLAT is None:
        nb = NCORES * RMAX * 2
        bo = np.empty(nb, np.int32)
        cf = np.empty(nb, np.int32)
        for c in range(NCORES):
            for r in range(RMAX):
                o_lo, o_hi = BUCKET_OFF[r]
                bo[(c * RMAX + r) * 2] = c * CAP + o_lo
                bo[(c * RMAX + r) * 2 + 1] = c * CAP + o_hi
                cf[(c * RMAX + r) * 2] = RANK_CAPS[r][0]
                cf[(c * RMAX + r) * 2 + 1] = RANK_CAPS[r][1]
        _BUCKET_OFF_FLAT, _CAPS_FLAT = bo, cf
    return _BUCKET_OFF_FLAT, _CAPS_FLAT


def _prep_edges(src, dst):
    """Lay out edges into per-core static buckets (round, src-half);
    returns per-core wrapped int16 gather/scatter arrays [16, CAP // 16]."""
    src = np.asarray(src, dtype=np.int64)
    dst32 = np.asarray(dst, dtype=np.int64).astype(np.int32)
    src_g = (src + 22 * (src // SHARD)).astype(np.int32)  # padded-global row
    hi = src_g >= SPLIT
    gidx_all = (src_g & 0x7FFF).astype(np.int16)  # == src_g - SPLIT for hi

    n = src.shape[0]
    # pass 1: group by dst to get each edge's rank within its dst node
    order1, deg, starts1 = _counting_sort(dst32, N)
    if int(deg.max()) > RMAX:
        raise ValueError(f"in-degree {int(deg.max())} exceeds RMAX={RMAX}")
    rank1 = (np.arange(n, dtype=np.int32)
             - np.repeat(starts1.astype(np.int32), deg))
    dst1 = dst32[order1]
    core1 = dst1 // SHARD
    hi1 = hi[order1]

    # pass 2: group by (core, rank, half) bucket
    key2 = ((core1 * RMAX + rank1) * 2 + hi1).astype(np.int32)
    nb = NCORES * RMAX * 2
    order2, counts2, starts2 = _counting_sort(key2, nb)

    bucket_off, caps_flat = _bucket_tables()
    if np.any(counts2 > caps_flat):
        b = int(np.argmax(counts2 - caps_flat))
        raise ValueError(f"bucket {b} overflow: {counts2[b]} > {caps_flat[b]}")

    rank2 = (np.arange(n, dtype=np.int32)
             - np.repeat(starts2.astype(np.int32), counts2))
    key2s = key2[order2]
    pos = bucket_off[key2s] + rank2
    perm = np.asarray(order1)[order2]   # original edge index per final slot

    gi_all = np.zeros(NCORES * CAP, np.int16)
    si_all = np.full(NCORES * CAP, DUMMY, np.int16)
    gi_all[pos] = gidx_all[perm]
    dstl = (dst1 - core1 * SHARD).astype(np.int16)
    si_all[pos] = dstl[order2]

    gw, sw = [], []
    for c in range(NCORES):
        gi = gi_all[c * CAP:(c + 1) * CAP]
        si = si_all[c * CAP:(c + 1) * CAP]
        gw.append(np.ascontiguousarray(gi.reshape(-1, 16).T))
        sw.append(np.ascontiguousarray(si.reshape(-1, 16).T))
    return gw, sw


def _pack_x(x):
    x = np.asarray(x, dtype=np.float32)
    xb = np.zeros((NCORES, SHARD_PAD, D), BF16_NP)
    xb[:, :SHARD] = x.reshape(NCORES, SHARD, D).astype(BF16_NP)
    return xb


def _pack_uv(U, V):
    return np.ascontiguousarray(
        np.stack([np.asarray(U, np.float32).T,
                  np.asarray(V, np.float32).T]).astype(BF16_NP))


def _unpack_out(outg_np):
    out = np.asarray(outg_np).reshape(NCORES, SHARD_PAD, D)[:, :SHARD]
    return np.ascontiguousarray(out.reshape(N, D).astype(np.float32))


_NC_CACHE = None
_FAST = None


def _build_fast():
    """Cached jit dispatch mirroring run_bass_via_pjrt, with device-side
    zero donation and single-shard output fetch."""
    import jax
    import jax.numpy as jnp
    from jax.sharding import Mesh, PartitionSpec, NamedSharding
    from jax.experimental.shard_map import shard_map
    from concourse.bass2jax import (_bass_exec_p, install_neuronx_cc_hook,
                                    partition_id_tensor)

    install_neuronx_cc_hook()
    nc = _NC_CACHE
    partition_name = nc.partition_id_tensor.name if nc.partition_id_tensor else None
    in_names, out_names, out_avals = [], [], []
    for alloc in nc.m.functions[0].allocations:
        if not isinstance(alloc, mybir.MemoryLocationSet):
            continue
        name = alloc.memorylocations[0].name
        if alloc.kind == "ExternalInput":
            if name != partition_name:
                in_names.append(name)
        elif alloc.kind == "ExternalOutput":
            out_names.append(name)
            out_avals.append(jax.core.ShapedArray(
                tuple(alloc.tensor_shape), mybir.dt.np(alloc.dtype)))
    n_params = len(in_names)
    n_outs = len(out_avals)
    all_names = in_names + out_names + ([partition_name] if partition_name else [])
    donate = tuple(range(n_params, n_params + n_outs))

    def _body(*args):
        operands = list(args)
        if partition_name is not None:
            operands.append(partition_id_tensor())
        return tuple(_bass_exec_p.bind(
            *operands, out_avals=tuple(out_avals), in_names=tuple(all_names),
            out_names=tuple(out_names), lowering_input_output_aliases=(),
            sim_require_finite=True, sim_require_nnan=True, nc=nc))

    devices = jax.devices()[:NCORES]
    mesh = Mesh(np.asarray(devices), ("core",))
    shd = NamedSharding(mesh, PartitionSpec("core"))
    sharded = jax.jit(
        shard_map(_body, mesh=mesh,
                  in_specs=(PartitionSpec("core"),) * (n_params + n_outs),
                  out_specs=(PartitionSpec("core"),) * n_outs, check_rep=False),
        donate_argnums=donate, keep_unused=True)

    zero_shapes = [(NCORES * a.shape[0], *a.shape[1:]) for a in out_avals]
    zero_dtypes = [a.dtype for a in out_avals]
    make_zeros = jax.jit(
        lambda: tuple(jnp.zeros(s, d) for s, d in zip(zero_shapes, zero_dtypes)),
        out_shardings=(shd,) * n_outs)

    return {"sharded": sharded, "make_zeros": make_zeros, "shd": shd,
            "in_names": in_names, "out_names": out_names, "jax": jax}


def _fast_call(dev_in):
    f = _FAST
    zeros = f["make_zeros"]()
    args = [dev_in[name] for name in f["in_names"]] + list(zeros)
    outs = f["sharded"](*args)
    i = f["out_names"].index("outg")
    arr = outs[i]
    shard0 = min(arr.addressable_shards, key=lambda s: s.index[0].start or 0)
    return np.asarray(shard0.data)


def kernel(x, src, dst, U, V):
    global _NC_CACHE, _FAST

    if _FAST is not None:
        import jax
        f = _FAST
        zeros = f["make_zeros"]()           # async, on device
        xb = _pack_x(x)
        xb_dev = jax.device_put(xb.reshape(NCORES * SHARD_PAD, D), f["shd"])
        gw, sw = _prep_edges(src, dst)      # overlaps the transfers above
        uvb = _pack_uv(U, V)
        dev_in = {
            "xb": xb_dev,
            "gidx": np.concatenate(gw, axis=0),
            "sidx": np.concatenate(sw, axis=0),
            "uv": np.concatenate([uvb] * NCORES, axis=0),
        }
        args = [dev_in[name] for name in f["in_names"]] + list(zeros)
        outs = f["sharded"](*args)
        i = f["out_names"].index("outg")
        arr = outs[i]
        shard0 = min(arr.addressable_shards, key=lambda s: s.index[0].start or 0)
        return _unpack_out(np.asarray(shard0.data))

    xb = _pack_x(x)
    gw, sw = _prep_edges(src, dst)
    uvb = _pack_uv(U, V)
    in_maps = [{"xb": xb[c], "gidx": gw[c], "sidx": sw[c], "uv": uvb}
               for c in range(NCORES)]

    if _NC_CACHE is None:
        _NC_CACHE = _build_nc()
    res = run_bass_kernel_spmd(_NC_CACHE, in_maps, core_ids=list(range(NCORES)))
    out = _unpack_out(res.results[0]["outg"])
    _FAST = _build_fast()
    return out
